# revision 34
# baseline (speedup 1.0000x reference)
"""MultiHeadAttention TRN2 kernel: batch-parallel across 8 NeuronCores.

Layout notes (per core, one batch element):
  xT   [768, 1024]  fp16  x[b] as (c, i) -- i = h*32+w token index
  wqk  [768, 1536]  fp16  permuted w_qkv columns: [q_h0 d0..63 (x8 scale), ...,
                          q_h11, k_h0, ..., k_h11]
  wv   [768, 768]   fp16  permuted v columns per head
  wp   [768, 768]   fp16  w_proj
  out  [768, 1024]  fp32  out^T (c', i) == (c, h, w) layout directly

Pipeline per head: S = q^T k (PE, [i,j] tiles) -> negmax via one
tensor_tensor_reduce (DVE, max over the two 512-halves, scale=-1 + min
accumulate) -> exp with bias + row-sum accumulator (ACT) into a per-head-half
P tile [128, 4096] -> ONE XBAR DMA transpose per half -> ctx^T accumulated
with V-stationary N=512 matmuls into a head-PAIR PSUM tile ([c,i] layout the
projection wants) -> 1/l row broadcast (Pool) -> fused evac*linv (DVE) ->
final projection straight from ctxT tiles.
"""
import numpy as np

HEADS, DH, DIM, N = 12, 64, 768, 1024
NB = 8  # batch == cores

_cache = {}


def _fix_drain_waits(nc, mybir, bass_rust):
    """This container's walrus has tight per-instruction sync-wait budgets
    (InstDrain tolerates none; Matmult only a couple). Hoist excess waits
    onto standalone event-semaphore wait instructions placed just before,
    chunked 4 waits apiece."""
    n = 0
    for f in nc.m.functions:
        for bb in f.blocks:
            new = []
            for ins in bb.instructions:
                si = ins.sync_info
                waits = list(si.on_wait) if si and si.on_wait else []
                limit = 0 if isinstance(ins, mybir.InstDrain) else 1
                if isinstance(ins, mybir.InstEventSemaphore):
                    limit = 2
                if len(waits) > limit:
                    keep, excess = waits[:limit], waits[limit:]
                    for c in range(0, len(excess), 2):
                        n += 1
                        ev = mybir.InstEventSemaphore(
                            name=f"{ins.name}-hoistw{n}", ins=[], outs=[])
                        ev.engine = ins.engine
                        ev.sync_info = bass_rust.SyncInfo(
                            on_wait=excess[c:c + 2], on_update=[])
                        nc.register_instruction(ev, overwrite=True)
                        new.append(ev)
                    si.on_wait = keep
                new.append(ins)
            bb.instructions[:] = new
    return n


def _build():
    import sys
    if "/opt/trn_rl_repo" not in sys.path:
        sys.path.insert(0, "/opt/trn_rl_repo")
    import bass_rust
    import concourse.bass as bass
    import concourse.mybir as mybir
    import concourse.tile as tile

    FP16, FP32 = mybir.dt.float16, mybir.dt.float32
    EXP = mybir.ActivationFunctionType.Exp
    AX = mybir.AxisListType.X
    MULT = mybir.AluOpType.mult
    KT = DIM // 128  # 6 contraction tiles
    HALVES = ((0, 512), (512, 1024))

    nc = bass.Bass()
    xT = nc.declare_dram_parameter("xT", [DIM, N], FP16, isOutput=False)
    wqk = nc.declare_dram_parameter("wqk", [DIM, 2 * DIM], FP16, isOutput=False)
    wv = nc.declare_dram_parameter("wv", [DIM, DIM], FP16, isOutput=False)
    wp = nc.declare_dram_parameter("wp", [DIM, DIM], FP16, isOutput=False)
    out = nc.declare_dram_parameter("out", [DIM, N], FP32, isOutput=True)
    # DRAM bounce rows for the 1/l broadcast (engines cannot replicate
    # across partitions; DMA broadcast only allows stride-0 on DRAM APs)
    ldram = nc.dram_tensor("ldram", (HEADS, N), FP16, kind="Internal")

    with tile.TileContext(nc) as tc:
        with (
            tc.tile_pool(name="win", bufs=1) as win,
            tc.tile_pool(name="qk", bufs=1) as qkp,
            tc.tile_pool(name="vp", bufs=1) as vp,
            tc.tile_pool(name="php", bufs=3) as php,
            tc.tile_pool(name="ptp", bufs=5) as ptp,
            tc.tile_pool(name="st", bufs=8) as st,
            tc.tile_pool(name="lp", bufs=3) as lp,
            tc.tile_pool(name="lbp", bufs=2) as lbp,
            tc.tile_pool(name="cxp", bufs=1) as cxp,
            tc.tile_pool(name="oup", bufs=2) as oup,
            tc.tile_pool(name="ps_mm", bufs=3, space="PSUM") as ps_mm,
            tc.tile_pool(name="ps_cx", bufs=1, space="PSUM") as ps_cx,
        ):
            # ---- resident loads (issued from the Scalar HWDGE queue so the
            # Sync queue is dedicated to the P^T transposes)
            xsb, wqksb, wvsb, wpsb = [], [], [], []
            for t in range(KT):
                xt = win.tile([128, N], FP16, tag=f"x{t}", name=f"x{t}")
                nc.sync.dma_start(xt[:], xT[t * 128:(t + 1) * 128, :])
                xsb.append(xt)
                wt = win.tile([128, 2 * DIM], FP16, tag=f"wqk{t}", name=f"wqk{t}")
                wqksb.append(wt)
            # earliest-needed q/k column blocks first, all on the sync queue
            # (idle during the prologue); later chunks are emitted mid-loop
            def load_wqk_chunk(lo, hi):
                for t in range(KT):
                    nc.sync.dma_start(wqksb[t][:, lo:hi],
                                      wqk[t * 128:(t + 1) * 128, lo:hi])

            load_wqk_chunk(768, 896)   # k pair 0
            load_wqk_chunk(0, 128)     # q pair 0
            load_wqk_chunk(896, 1024)  # k pair 1
            load_wqk_chunk(128, 256)   # q pair 1
            for t in range(KT):
                vt = win.tile([128, DIM], FP16, tag=f"wv{t}", name=f"wv{t}")
                nc.sync.dma_start(vt[:], wv[t * 128:(t + 1) * 128, :])
                wvsb.append(vt)
            for t in range(KT):
                pt_ = win.tile([128, DIM], FP16, tag=f"wp{t}", name=f"wp{t}")
                wpsb.append(pt_)

            # persistent small tiles
            linv = win.tile([128, 128], FP16, tag="linv", name="linv")
            nc.gpsimd.memset(linv[:], 0.0)  # cols 8:128 stay 0 (uninit guard)
            # 1/l transposed: linvT[it, ip] = linv[ip, it] (rows 0:8 valid)
            linvT = win.tile([128, 128], FP16, tag="linvT", name="linvT")
            ctxT = [cxp.tile([128, N], FP16, tag=f"ctxT{t}", name=f"ctxT{t}")
                    for t in range(KT)]

            qksb = [None] * 12
            vsb = [None] * 8
            PH = {}     # (h, half) -> P tile [128, 4096]
            PT = {}     # (h, half) -> P^T tile [128, 4096]
            psc = {}    # pair t -> ctx^T psum [128, 1024]
            lof = {}    # h -> l accum tile [128, 8] fp32
            lb = {}     # pair t -> 1/l broadcast [128, 1024] fp16

            def emit_qk_block(m):
                ps = ps_mm.tile([128, N], FP32, tag="mm", name="mm")
                for t in range(KT):
                    for lo, hi in HALVES:
                        nc.tensor.matmul(
                            ps[:, lo:hi],
                            wqksb[t][:, m * 128:(m + 1) * 128],
                            xsb[t][:, lo:hi],
                            start=(t == 0), stop=(t == KT - 1),
                        )
                qt = qkp.tile([128, N], FP16, tag=f"qk{m}", name=f"qk{m}")
                nc.vector.tensor_copy(qt[:], ps[:])
                qksb[m] = qt

            def emit_v_tile(j):
                ps = ps_mm.tile([128, N], FP32, tag="mm", name="mm")
                for t in range(KT):
                    nc.tensor.matmul(ps[:, 0:512], xsb[t][:, j * 128:(j + 1) * 128],
                                     wvsb[t][:, 0:512],
                                     start=(t == 0), stop=(t == KT - 1))
                    nc.tensor.matmul(ps[:, 512:768], xsb[t][:, j * 128:(j + 1) * 128],
                                     wvsb[t][:, 512:768],
                                     start=(t == 0), stop=(t == KT - 1))
                vt = vp.tile([128, DIM], FP16, tag=f"v{j}", name=f"v{j}")
                nc.scalar.copy(vt[:], ps[:, 0:DIM])
                vsb[j] = vt

            def emit_S(h, half, itl):
                it = half * 4 + itl
                prow = (h % 2) * 64
                q_ap = qksb[h // 2][prow:prow + 64, :]
                k_ap = qksb[6 + h // 2][prow:prow + 64, :]
                ps = ps_mm.tile([128, N], FP32, tag="mm", name="mm")
                for lo, hi in HALVES:
                    nc.tensor.matmul(ps[:, lo:hi],
                                     q_ap[:, it * 128:(it + 1) * 128],
                                     k_ap[:, lo:hi], start=True, stop=True)
                negmax = st.tile([128, 1], FP32, tag="negmax", name="negmax")
                nc.vector.tensor_reduce(negmax[:], ps[:], axis=AX,
                                        op=mybir.AluOpType.max, negate=True)
                nc.scalar.activation(
                    PH[(h, half)][:, itl * N:(itl + 1) * N], ps[:], EXP,
                    bias=negmax[:], scale=1.0,
                    accum_out=lof[h][:, it:it + 1])

            def emit_transpose_chunk(h, half, itl):
                # per-it chunk: starts as soon as its exp lands, smoothing the
                # sync queue and finishing each half ~4us earlier
                if (h, half) not in PT:
                    PT[(h, half)] = ptp.tile([128, 4 * N], FP16, tag="pt",
                                             name="pt")
                pt = PT[(h, half)]
                nc.sync.dma_start_transpose(
                    pt[:, itl * N:(itl + 1) * N].rearrange(
                        "p (b c) -> p b c", c=128),
                    PH[(h, half)][:, itl * N:(itl + 1) * N])
                if itl == 3:
                    PH.pop((h, half))

            def emit_ctx(h, half, jbs):
                t = h // 2
                if t not in psc:
                    psc[t] = ps_cx.tile([128, N], FP32, tag="cx", name="cx")
                prow = (h % 2) * 64
                pt4 = PT[(h, half)][:, :].rearrange(
                    "p (i b c) -> p i b c", b=8, c=128)
                for jb in jbs:
                    nc.tensor.matmul(
                        psc[t][prow:prow + 64, half * 512:(half + 1) * 512],
                        vsb[jb][:, h * DH:(h + 1) * DH],
                        pt4[:, :, jb, :],
                        start=(jb == 0), stop=(jb == 7),
                        skip_group_check=True)
                if jbs[-1] == 7:
                    PT.pop((h, half))

            def emit_lpath(h):
                t = h // 2
                with nc.allow_low_precision(reason="1/l in fp16: l in [1,1e3]"):
                    nc.vector.reciprocal(linv[:, 0:8], lof[h][:, 0:8])
                nc.sync.dma_start_transpose(linvT[:, :], linv[:, :])
                nc.sync.dma_start(
                    ldram[h:h + 1, :].rearrange("a (t c) -> a t c", c=128),
                    linvT[0:8, 0:128])
                if t not in lb:
                    lb[t] = lbp.tile([128, N], FP16, tag="lb", name="lb")
                prow = (h % 2) * 64
                nc.sync.dma_start(lb[t][prow:prow + 64, :],
                                  ldram[h:h + 1, :].to_broadcast((64, N)))
                del lof[h]

            def emit_evac(t):
                nc.vector.tensor_tensor(ctxT[t][:, :], psc.pop(t)[:, :],
                                        lb.pop(t)[:, :], op=MULT)

            # ---- prologue: first q/k pair
            emit_qk_block(6)
            emit_qk_block(0)

            # ---- projection helpers: partial accumulation over k-tiles
            ps_proj = {}

            def emit_proj(cp, ts, stop):
                if cp not in ps_proj:
                    ps_proj[cp] = ps_mm.tile([128, N], FP32, tag="mm",
                                             name="mm")
                ps = ps_proj[cp]
                for t in ts:
                    for lo, hi in HALVES:
                        nc.tensor.matmul(ps[:, lo:hi],
                                         wpsb[t][:, cp * 128:(cp + 1) * 128],
                                         ctxT[t][:, lo:hi],
                                         start=(t == 0),
                                         stop=(stop and t == ts[-1]),
                                         skip_group_check=True)

            def emit_proj_out(cp):
                ot = oup.tile([128, N], FP32, tag="osb", name="osb")
                nc.scalar.copy(ot[:], ps_proj.pop(cp)[:])
                nc.scalar.dma_start(out[cp * 128:(cp + 1) * 128, :], ot[:])

            # ---- main pipeline: S(h) while ctx(h-2) chases
            for step in range(14):
                h, hc = step, step - 2
                if h == 0:
                    load_wqk_chunk(1024, 1536)  # k pairs 2-5
                elif h == 1:
                    load_wqk_chunk(256, 768)    # q pairs 2-5
                elif h == 2:
                    for t in range(KT):
                        nc.sync.dma_start(wpsb[t][:],
                                          wp[t * 128:(t + 1) * 128, :])
                for half in (0, 1):
                    if h < 12:
                        if half == 0:
                            lof[h] = lp.tile([128, 8], FP32, tag="l", name="l")
                        PH[(h, half)] = php.tile([128, 4 * N], FP16, tag="p",
                                                 name="p")
                        emit_S(h, half, 0)
                        emit_S(h, half, 1)
                        emit_transpose_chunk(h, half, 0)
                    if hc >= 0:
                        emit_ctx(hc, half, (0, 1, 2, 3))
                    if h < 12:
                        emit_S(h, half, 2)
                        emit_transpose_chunk(h, half, 1)
                    if hc >= 0:
                        emit_ctx(hc, half, (4, 5, 6, 7))
                        if half == 1 and hc % 2 == 1:
                            emit_evac(hc // 2)
                    if h < 12:
                        emit_S(h, half, 3)
                        emit_transpose_chunk(h, half, 2)
                        emit_transpose_chunk(h, half, 3)
                    # PE fillers: spread next q/k pair + V tiles evenly
                    if half == 1 and h < 10:
                        p = h // 2 + 1
                        if h % 2 == 0:
                            emit_qk_block(6 + p)
                        else:
                            emit_qk_block(p)
                    if h in (0, 1):
                        for j in (2 * h, 2 * h + 1) if half == 0 else (
                                2 * h + 4, 2 * h + 5):
                            emit_v_tile(j)
                    if h == 12:
                        # S is done; fill PE with early projection partials
                        for cp in (0, 1, 2) if half == 0 else ():
                            emit_proj(cp, (0, 1, 2), stop=False)
                        for cp in (0, 1, 2) if half == 1 else ():
                            emit_proj(cp, (3, 4), stop=False)
                if h < 12:
                    emit_lpath(h)

            # ---- tail: finish projections, evac + store on ACT
            for cp in (0, 1, 2):
                emit_proj(cp, (5,), stop=True)
                emit_proj_out(cp)
            for cp in (3, 4, 5):
                emit_proj(cp, (0, 1, 2, 3, 4, 5), stop=True)
                emit_proj_out(cp)

    _fix_drain_waits(nc, mybir, bass_rust)
    return nc


def _prep(w_qkv, w_proj):
    r = np.arange(DIM)
    head, d = r // DH, r % DH
    qcols = d * (3 * HEADS) + 0 * HEADS + head
    kcols = d * (3 * HEADS) + 1 * HEADS + head
    vcols = d * (3 * HEADS) + 2 * HEADS + head
    w = np.asarray(w_qkv, np.float32)
    wqk = np.concatenate([w[:, qcols] * np.float32(DH ** 0.5), w[:, kcols]],
                         axis=1).astype(np.float16)
    wv = np.ascontiguousarray(w[:, vcols]).astype(np.float16)
    wp = np.asarray(w_proj, np.float32).astype(np.float16)
    return wqk, wv, wp


def _run(x, w_qkv, w_proj, **spmd_kwargs):
    import sys
    if "/opt/trn_rl_repo" not in sys.path:
        sys.path.insert(0, "/opt/trn_rl_repo")
    from concourse.bass_utils import run_bass_kernel_spmd

    if "nc" not in _cache:
        _cache["nc"] = _build()
    nc = _cache["nc"]

    x = np.asarray(x, np.float32)
    wqk, wv, wp = _prep(w_qkv, w_proj)
    xTs = x.reshape(NB, DIM, N).astype(np.float16)

    in_maps = [
        {"xT": xTs[b], "wqk": wqk, "wv": wv, "wp": wp} for b in range(NB)
    ]
    res = run_bass_kernel_spmd(nc, in_maps, list(range(NB)), **spmd_kwargs)
    outs = np.stack([np.asarray(res.results[b]["out"], np.float32)
                     for b in range(NB)])
    return outs.reshape(NB, DIM, 32, 32), res


def kernel(x, w_qkv, w_proj):
    return _run(x, w_qkv, w_proj)[0]


# revision 36
# speedup vs baseline: 1.0565x; 1.0565x over previous
"""MultiHeadAttention TRN2 kernel: batch-parallel across 8 NeuronCores.

Layout notes (per core, one batch element):
  xT   [768, 1024]  fp16  x[b] as (c, i) -- i = h*32+w token index
  wqk  [768, 1536]  fp16  permuted w_qkv columns: [q_h0 d0..63 (x8 scale), ...,
                          q_h11, k_h0, ..., k_h11]
  wv   [768, 768]   fp16  permuted v columns per head
  wp   [768, 768]   fp16  w_proj
  out  [768, 1024]  fp32  out^T (c', i) == (c, h, w) layout directly

Pipeline per head: S = q^T k (PE, [i,j] tiles) -> negmax via one
tensor_tensor_reduce (DVE, max over the two 512-halves, scale=-1 + min
accumulate) -> exp with bias + row-sum accumulator (ACT) into a per-head-half
P tile [128, 4096] -> ONE XBAR DMA transpose per half -> ctx^T accumulated
with V-stationary N=512 matmuls into a head-PAIR PSUM tile ([c,i] layout the
projection wants) -> 1/l row broadcast (Pool) -> fused evac*linv (DVE) ->
final projection straight from ctxT tiles.
"""
import numpy as np

HEADS, DH, DIM, N = 12, 64, 768, 1024
NB = 8  # batch == cores

_cache = {}


def _fix_drain_waits(nc, mybir, bass_rust):
    """This container's walrus has tight per-instruction sync-wait budgets
    (InstDrain tolerates none; Matmult only a couple). Hoist excess waits
    onto standalone event-semaphore wait instructions placed just before,
    chunked 4 waits apiece."""
    n = 0
    for f in nc.m.functions:
        for bb in f.blocks:
            new = []
            for ins in bb.instructions:
                si = ins.sync_info
                waits = list(si.on_wait) if si and si.on_wait else []
                limit = 0 if isinstance(ins, mybir.InstDrain) else 1
                if isinstance(ins, mybir.InstEventSemaphore):
                    limit = 2
                if len(waits) > limit:
                    keep, excess = waits[:limit], waits[limit:]
                    for c in range(0, len(excess), 2):
                        n += 1
                        ev = mybir.InstEventSemaphore(
                            name=f"{ins.name}-hoistw{n}", ins=[], outs=[])
                        ev.engine = ins.engine
                        ev.sync_info = bass_rust.SyncInfo(
                            on_wait=excess[c:c + 2], on_update=[])
                        nc.register_instruction(ev, overwrite=True)
                        new.append(ev)
                    si.on_wait = keep
                new.append(ins)
            bb.instructions[:] = new
    return n


def _build():
    import sys
    if "/opt/trn_rl_repo" not in sys.path:
        sys.path.insert(0, "/opt/trn_rl_repo")
    import bass_rust
    import concourse.bass as bass
    import concourse.mybir as mybir
    import concourse.tile as tile

    FP16, FP32 = mybir.dt.float16, mybir.dt.float32
    EXP = mybir.ActivationFunctionType.Exp
    AX = mybir.AxisListType.X
    MULT = mybir.AluOpType.mult
    KT = DIM // 128  # 6 contraction tiles
    HALVES = ((0, 512), (512, 1024))

    nc = bass.Bass()
    xT = nc.declare_dram_parameter("xT", [DIM, N], FP16, isOutput=False)
    wqk = nc.declare_dram_parameter("wqk", [DIM, 2 * DIM], FP16, isOutput=False)
    wv = nc.declare_dram_parameter("wv", [DIM, DIM], FP16, isOutput=False)
    wp = nc.declare_dram_parameter("wp", [DIM, DIM], FP16, isOutput=False)
    out = nc.declare_dram_parameter("out", [DIM, N], FP32, isOutput=True)
    # DRAM bounce rows for the 1/l broadcast (engines cannot replicate
    # across partitions; DMA broadcast only allows stride-0 on DRAM APs)
    ldram = nc.dram_tensor("ldram", (HEADS, N), FP16, kind="Internal")

    with tile.TileContext(nc) as tc:
        with (
            tc.tile_pool(name="win", bufs=1) as win,
            tc.tile_pool(name="qk", bufs=1) as qkp,
            tc.tile_pool(name="vp", bufs=1) as vp,
            tc.tile_pool(name="php", bufs=3) as php,
            tc.tile_pool(name="ptp", bufs=5) as ptp,
            tc.tile_pool(name="st", bufs=8) as st,
            tc.tile_pool(name="lp", bufs=3) as lp,
            tc.tile_pool(name="lbp", bufs=2) as lbp,
            tc.tile_pool(name="cxp", bufs=1) as cxp,
            tc.tile_pool(name="oup", bufs=2) as oup,
            tc.tile_pool(name="ps_mm", bufs=3, space="PSUM") as ps_mm,
            tc.tile_pool(name="ps_cx", bufs=1, space="PSUM") as ps_cx,
        ):
            # ---- resident loads (issued from the Scalar HWDGE queue so the
            # Sync queue is dedicated to the P^T transposes)
            xsb, wqksb, wvsb, wpsb = [], [], [], []
            for t in range(KT):
                xt = win.tile([128, N], FP16, tag=f"x{t}", name=f"x{t}")
                nc.sync.dma_start(xt[:], xT[t * 128:(t + 1) * 128, :])
                xsb.append(xt)
                wt = win.tile([128, 2 * DIM], FP16, tag=f"wqk{t}", name=f"wqk{t}")
                wqksb.append(wt)
            # earliest-needed q/k column blocks first, all on the sync queue
            # (idle during the prologue); later chunks are emitted mid-loop
            def load_wqk_chunk(lo, hi):
                for t in range(KT):
                    nc.sync.dma_start(wqksb[t][:, lo:hi],
                                      wqk[t * 128:(t + 1) * 128, lo:hi])

            load_wqk_chunk(768, 896)   # k pair 0
            load_wqk_chunk(0, 128)     # q pair 0
            load_wqk_chunk(896, 1024)  # k pair 1
            load_wqk_chunk(128, 256)   # q pair 1
            for t in range(KT):
                vt = win.tile([128, DIM], FP16, tag=f"wv{t}", name=f"wv{t}")
                nc.sync.dma_start(vt[:], wv[t * 128:(t + 1) * 128, :])
                wvsb.append(vt)
            for t in range(KT):
                pt_ = win.tile([128, DIM], FP16, tag=f"wp{t}", name=f"wp{t}")
                wpsb.append(pt_)

            # persistent small tiles
            linv = win.tile([128, 128], FP16, tag="linv", name="linv")
            nc.gpsimd.memset(linv[:], 0.0)  # cols 8:128 stay 0 (uninit guard)
            # 1/l transposed: linvT[it, ip] = linv[ip, it] (rows 0:8 valid)
            linvT = win.tile([128, 128], FP16, tag="linvT", name="linvT")
            ctxT = [cxp.tile([128, N], FP16, tag=f"ctxT{t}", name=f"ctxT{t}")
                    for t in range(KT)]

            qksb = [None] * 12
            vsb = [None] * 8
            PH = {}     # (h, half) -> P tile [128, 4096]
            PT = {}     # (h, half) -> P^T tile [128, 4096]
            psc = {}    # pair t -> ctx^T psum [128, 1024]
            lof = {}    # h -> l accum tile [128, 8] fp32
            lb = {}     # pair t -> 1/l broadcast [128, 1024] fp16

            def emit_qk_block(m):
                ps = ps_mm.tile([128, N], FP32, tag="mm", name="mm")
                for t in range(KT):
                    for lo, hi in HALVES:
                        nc.tensor.matmul(
                            ps[:, lo:hi],
                            wqksb[t][:, m * 128:(m + 1) * 128],
                            xsb[t][:, lo:hi],
                            start=(t == 0), stop=(t == KT - 1),
                        )
                qt = qkp.tile([128, N], FP16, tag=f"qk{m}", name=f"qk{m}")
                nc.vector.tensor_copy(qt[:], ps[:])
                qksb[m] = qt

            def emit_v_tile(j):
                ps = ps_mm.tile([128, N], FP32, tag="mm", name="mm")
                for t in range(KT):
                    nc.tensor.matmul(ps[:, 0:512], xsb[t][:, j * 128:(j + 1) * 128],
                                     wvsb[t][:, 0:512],
                                     start=(t == 0), stop=(t == KT - 1))
                    nc.tensor.matmul(ps[:, 512:768], xsb[t][:, j * 128:(j + 1) * 128],
                                     wvsb[t][:, 512:768],
                                     start=(t == 0), stop=(t == KT - 1))
                vt = vp.tile([128, DIM], FP16, tag=f"v{j}", name=f"v{j}")
                nc.scalar.copy(vt[:], ps[:, 0:DIM])
                vsb[j] = vt

            def emit_S(h, half, itl):
                it = half * 4 + itl
                prow = (h % 2) * 64
                q_ap = qksb[h // 2][prow:prow + 64, :]
                k_ap = qksb[6 + h // 2][prow:prow + 64, :]
                ps = ps_mm.tile([128, N], FP32, tag="mm", name="mm")
                for lo, hi in HALVES:
                    nc.tensor.matmul(ps[:, lo:hi],
                                     q_ap[:, it * 128:(it + 1) * 128],
                                     k_ap[:, lo:hi], start=True, stop=True)
                negmax = st.tile([128, 1], FP32, tag="negmax", name="negmax")
                nc.vector.tensor_reduce(negmax[:], ps[:], axis=AX,
                                        op=mybir.AluOpType.max, negate=True)
                nc.scalar.activation(
                    PH[(h, half)][:, itl * N:(itl + 1) * N], ps[:], EXP,
                    bias=negmax[:], scale=1.0,
                    accum_out=lof[h][:, it:it + 1])

            def emit_transpose(h, half):
                pt = ptp.tile([128, 4 * N], FP16, tag="pt", name="pt")
                nc.sync.dma_start_transpose(
                    pt[:, :].rearrange("p (m c) -> p m c", c=128),
                    PH[(h, half)][:, :])
                PT[(h, half)] = pt
                PH.pop((h, half))

            def emit_ctx(h, half, jbs):
                t = h // 2
                if t not in psc:
                    psc[t] = ps_cx.tile([128, N], FP32, tag="cx", name="cx")
                prow = (h % 2) * 64
                pt4 = PT[(h, half)][:, :].rearrange(
                    "p (i b c) -> p i b c", b=8, c=128)
                for jb in jbs:
                    nc.tensor.matmul(
                        psc[t][prow:prow + 64, half * 512:(half + 1) * 512],
                        vsb[jb][:, h * DH:(h + 1) * DH],
                        pt4[:, :, jb, :],
                        start=(jb == 0), stop=(jb == 7),
                        skip_group_check=True)
                if jbs[-1] == 7:
                    PT.pop((h, half))

            def emit_lpath(h):
                t = h // 2
                with nc.allow_low_precision(reason="1/l in fp16: l in [1,1e3]"):
                    nc.vector.reciprocal(linv[:, 0:8], lof[h][:, 0:8])
                nc.sync.dma_start_transpose(linvT[:, :], linv[:, :])
                nc.sync.dma_start(
                    ldram[h:h + 1, :].rearrange("a (t c) -> a t c", c=128),
                    linvT[0:8, 0:128])
                if t not in lb:
                    lb[t] = lbp.tile([128, N], FP16, tag="lb", name="lb")
                prow = (h % 2) * 64
                nc.sync.dma_start(lb[t][prow:prow + 64, :],
                                  ldram[h:h + 1, :].to_broadcast((64, N)))
                del lof[h]

            def emit_evac(t):
                nc.vector.tensor_tensor(ctxT[t][:, :], psc.pop(t)[:, :],
                                        lb.pop(t)[:, :], op=MULT)

            # ---- prologue: first q/k pair
            emit_qk_block(6)
            emit_qk_block(0)

            # ---- projection helpers: partial accumulation over k-tiles
            ps_proj = {}

            def emit_proj(cp, ts, stop):
                if cp not in ps_proj:
                    ps_proj[cp] = ps_mm.tile([128, N], FP32, tag="mm",
                                             name="mm")
                ps = ps_proj[cp]
                for t in ts:
                    for lo, hi in HALVES:
                        nc.tensor.matmul(ps[:, lo:hi],
                                         wpsb[t][:, cp * 128:(cp + 1) * 128],
                                         ctxT[t][:, lo:hi],
                                         start=(t == 0),
                                         stop=(stop and t == ts[-1]),
                                         skip_group_check=True)

            def emit_proj_out(cp):
                ot = oup.tile([128, N], FP32, tag="osb", name="osb")
                nc.scalar.copy(ot[:], ps_proj.pop(cp)[:])
                nc.scalar.dma_start(out[cp * 128:(cp + 1) * 128, :], ot[:])

            # ---- main pipeline: S(h) while ctx(h-2) chases
            for step in range(14):
                h, hc = step, step - 2
                if h == 0:
                    load_wqk_chunk(1024, 1536)  # k pairs 2-5
                elif h == 1:
                    load_wqk_chunk(256, 768)    # q pairs 2-5
                elif h == 2:
                    for t in range(KT):
                        nc.sync.dma_start(wpsb[t][:],
                                          wp[t * 128:(t + 1) * 128, :])
                # S its first, with ctx matmuls woven between them so the PE
                # queue never injects ctx latency into the softmax chain
                ctx_q = []
                if hc >= 0:
                    ctx_q = [(hc, 0, jb) for jb in range(8)] + \
                            [(hc, 1, jb) for jb in range(8)]

                def drain_ctx(k):
                    while k > 0 and ctx_q:
                        c_h, c_half, c_jb = ctx_q.pop(0)
                        emit_ctx(c_h, c_half, (c_jb,))
                        k -= 1

                if h < 12:
                    lof[h] = lp.tile([128, 8], FP32, tag="l", name="l")
                    for half in (0, 1):
                        PH[(h, half)] = php.tile([128, 4 * N], FP16, tag="p",
                                                 name="p")
                    emit_S(h, 0, 0)
                    emit_S(h, 0, 1)
                    emit_S(h, 0, 2)
                    for itl in (3, 4, 5, 6, 7):
                        drain_ctx(2)
                        emit_S(h, itl // 4, itl % 4)
                        if itl == 3:
                            emit_transpose(h, 0)
                    emit_lpath(h)
                    emit_transpose(h, 1)
                drain_ctx(16)
                if hc >= 0 and hc % 2 == 1:
                    emit_evac(hc // 2)
                # PE fillers at step end: next q/k pair + V tiles + proj
                if h < 10:
                    p = h // 2 + 1
                    emit_qk_block((6 + p) if h % 2 == 0 else p)
                if h in (0, 1):
                    for j in (4 * h, 4 * h + 1, 4 * h + 2, 4 * h + 3):
                        emit_v_tile(j)
                if h == 12:
                    # S is done; fill PE with early projection partials
                    for cp in (0, 1, 2):
                        emit_proj(cp, (0, 1, 2, 3, 4), stop=False)

            # ---- tail: finish projections, evac + store on ACT
            for cp in (0, 1, 2):
                emit_proj(cp, (5,), stop=True)
                emit_proj_out(cp)
            for cp in (3, 4, 5):
                emit_proj(cp, (0, 1, 2, 3, 4, 5), stop=True)
                emit_proj_out(cp)

    _fix_drain_waits(nc, mybir, bass_rust)
    return nc


def _prep(w_qkv, w_proj):
    r = np.arange(DIM)
    head, d = r // DH, r % DH
    qcols = d * (3 * HEADS) + 0 * HEADS + head
    kcols = d * (3 * HEADS) + 1 * HEADS + head
    vcols = d * (3 * HEADS) + 2 * HEADS + head
    w = np.asarray(w_qkv, np.float32)
    wqk = np.concatenate([w[:, qcols] * np.float32(DH ** 0.5), w[:, kcols]],
                         axis=1).astype(np.float16)
    wv = np.ascontiguousarray(w[:, vcols]).astype(np.float16)
    wp = np.asarray(w_proj, np.float32).astype(np.float16)
    return wqk, wv, wp


def _run(x, w_qkv, w_proj, **spmd_kwargs):
    import sys
    if "/opt/trn_rl_repo" not in sys.path:
        sys.path.insert(0, "/opt/trn_rl_repo")
    from concourse.bass_utils import run_bass_kernel_spmd

    if "nc" not in _cache:
        _cache["nc"] = _build()
    nc = _cache["nc"]

    x = np.asarray(x, np.float32)
    wqk, wv, wp = _prep(w_qkv, w_proj)
    xTs = x.reshape(NB, DIM, N).astype(np.float16)

    in_maps = [
        {"xT": xTs[b], "wqk": wqk, "wv": wv, "wp": wp} for b in range(NB)
    ]
    res = run_bass_kernel_spmd(nc, in_maps, list(range(NB)), **spmd_kwargs)
    outs = np.stack([np.asarray(res.results[b]["out"], np.float32)
                     for b in range(NB)])
    return outs.reshape(NB, DIM, 32, 32), res


def kernel(x, w_qkv, w_proj):
    return _run(x, w_qkv, w_proj)[0]
